# revision 1
# baseline (speedup 1.0000x reference)
"""Trainium2 Bass kernel for nn_BatchedVQLAMDecoder (8-core SPMD).

Sharding: mod-8 interleave of the 4120-token sequence (8 chunks of 515,
padded to 520). Core k owns padded rows p == k (mod 8) of every chunk:
local row l = 65*c + jj  <->  global padded row 520*c + 8*jj + k.
Block-causal mask at chunk granularity => rows of chunk c attend to all
of chunks 0..c' (c' <= c) with NO masking; every core runs the identical
program on different data (uniform SPMD, balanced causal work).

Per layer: local LN1 -> qkv^T -> AllGather of (K^T, V) fp16 -> attention
col-chunk loop (S^T = K^T.T @ Q^T, exp on ACT, o^T += [V|1].T @ E) ->
normalize via ones-column denominators -> out-proj -> LN2 -> MLP (g^T
form) -> residuals. All matmul operands fp16, accumulation f32.
"""

import os
import threading

import numpy as np

import concourse.bass as bass
import concourse.tile as tile
from concourse import bacc, mybir
from concourse.bass_utils import run_bass_kernel_spmd
from concourse.bass_interp import get_hw_module

F32 = mybir.dt.float32
F16 = mybir.dt.float16
AF = mybir.ActivationFunctionType

T, N, D, CDim, E, H, DEPTH = 8, 256, 1024, 128, 512, 8, 3
Dh = E // H                   # 64
CHUNK = 2 * N + 3             # 515
JJ = 65                       # local rows per (core, chunk)
PC = 8 * JJ                   # padded chunk = 520
LR = T * JJ                   # local rows per core = 520
LRP = 528                     # padded to mult-16 for dma transpose
NC_ = 8
KVLEN = E * PC + PC * E       # fp16 elems in one rank's AG payload
SCALE = 1.0 / np.sqrt(Dh)

# token-major row tiles over the 528 padded local rows
RT = [(0, 128), (128, 128), (256, 128), (384, 128), (512, 16)]
# real-row counts per row tile (rows 520..527 are pad)
RT_REAL = [128, 128, 128, 128, 8]
# col subtiles over 520 gathered cols of one chunk
CS = [(0, 128), (128, 128), (256, 128), (384, 128), (512, 8)]


def _nsplits(n):
    out = []
    o = 0
    while o < n:
        w = min(512, n - o)
        out.append((o, w))
        o += w
    return out


def _row_spans(lo, cnt):
    """Split local rows [lo, lo+cnt) by the 128-partition tile grid.
    Yields (tile_idx, part_offset_in_tile, count, offset_in_range)."""
    out = []
    done = 0
    while done < cnt:
        g = lo + done
        t = g // 128
        po = g % 128
        c = min(128 - po, cnt - done)
        out.append((t, po, c, done))
        done += c
    return out


def _ln_layer(nc, pools, src_tiles, g_bc, b_bc, eps_t, out_dtype, affine=True):
    """LayerNorm over E=512 on token-major tiles. Returns new fp16 tiles."""
    work = pools["work"]
    outs = []
    for ti, (lo, cnt) in enumerate(RT):
        x = src_tiles[ti]
        stats = work.tile([128, 6], F32, tag="ln_stats")
        nc.vector.bn_stats(out=stats[:cnt], in_=x[:cnt])
        mv = work.tile([128, 2], F32, tag="ln_mv")
        nc.vector.bn_aggr(out=mv[:cnt], in_=stats[:cnt])
        rs = work.tile([128, 1], F32, tag="ln_rs")
        nc.scalar.activation(out=rs[:cnt], in_=mv[:cnt, 1:2], func=AF.Sqrt,
                             bias=eps_t[:cnt], scale=1.0)
        nc.vector.reciprocal(out=rs[:cnt], in_=rs[:cnt])
        y = work.tile([128, E], F32, tag="ln_y")
        nc.vector.tensor_scalar(out=y[:cnt], in0=x[:cnt],
                                scalar1=mv[:cnt, 0:1], scalar2=rs[:cnt],
                                op0=mybir.AluOpType.subtract,
                                op1=mybir.AluOpType.mult)
        h = work.tile([128, E], out_dtype, tag="ln_h")
        if affine:
            nc.vector.tensor_mul(out=y[:cnt], in0=y[:cnt], in1=g_bc[:cnt])
            nc.vector.tensor_add(out=h[:cnt], in0=y[:cnt], in1=b_bc[:cnt])
        else:
            nc.vector.tensor_copy(out=h[:cnt], in_=y[:cnt])
        outs.append((h, cnt))
    return outs


def _transpose_rows(nc, pools, h_tiles, tag):
    """h (token-major fp16 [RT,512]) -> hT[e] fp16 [128, LRP] (e: E/128)."""
    work = pools["work"]
    hT = [work.tile([128, LRP], F16, tag=f"{tag}{e}", name=f"{tag}{e}", bufs=2) for e in range(4)]
    for ti, (lo, cnt) in enumerate(RT):
        h, real = h_tiles[ti]
        pcnt = 128 if ti < 4 else 16
        # pad rows (real..pcnt) hold garbage-but-finite data; transposed
        # cols beyond 520 are never read.
        for e in range(4):
            nc.scalar.dma_start_transpose(
                out=hT[e][:, lo:lo + pcnt],
                in_=h[:pcnt, 128 * e:128 * (e + 1)])
    return hT


def build_program():
    nc = bacc.Bacc("TRN2", target_bir_lowering=False, debug=False,
                   num_devices=NC_)

    # ---------------- DRAM I/O ----------------
    di = {}

    def inp(name, shape, dt):
        di[name] = nc.dram_tensor(name, list(shape), dt,
                                  kind="ExternalInput").ap()
        return di[name]

    zT_d = inp("zT", [D, 2 * CDim], F16)             # [1024, 256]
    posz_d = inp("posz", [2 * CDim, E], F32)         # [256, 512]
    rest_d = inp("rest", [T * 33, E], F32)           # [264, 512]
    vones_d = inp("vones", [PC, 8], F16)
    rmask_d = inp("rowmask", [LRP, 1], F32)
    pw_d = inp("patch_w", [D, E], F16)
    pb_d = inp("patch_b", [E], F32)
    qkw_d = inp("qk_w", [DEPTH, E, 2 * E], F16)
    vw_d = inp("v_w", [DEPTH, E, E], F16)
    ow_d = inp("o_w", [DEPTH, E, E], F16)
    w1_d = inp("w1", [DEPTH, E, 4 * E], F16)
    w2_d = inp("w2", [DEPTH, 4 * E, E], F16)
    qkbp_d = inp("qkb_p", [DEPTH, 128, 8], F32)
    vb_d = inp("v_b", [DEPTH, E], F32)
    outb_d = inp("out_b", [DEPTH, E], F32)
    b1p_d = inp("b1_p", [DEPTH, 128, 16], F32)
    b2_d = inp("b2", [DEPTH, E], F32)
    ln1g_d = inp("ln1g", [DEPTH, E], F32)
    ln1b_d = inp("ln1b", [DEPTH, E], F32)
    ln2g_d = inp("ln2g", [DEPTH, E], F32)
    ln2b_d = inp("ln2b", [DEPTH, E], F32)
    ng_d = inp("norm_g", [E], F32)
    nb_d = inp("norm_b", [E], F32)
    opw_d = inp("oproj_w", [E, D], F16)
    opb_d = inp("oproj_b", [D], F32)

    out_d = nc.dram_tensor("out", [T * 33, D], F32, kind="ExternalOutput").ap()

    dn_dram = nc.dram_tensor("dn_dram", [H, PC], F32).ap()
    # per-chunk AG buffers: each rank ships K^T[:, 65c:65c+65] + V[65c:65c+65, :]
    CKV = E * JJ + JJ * E
    kv_locq, kt_locq, v_locq, ktgq, vgq = [], [], [], [], []
    for qi in range(T):
        kl = nc.dram_tensor(f"kv_loc{qi}", [CKV], F16).ap()
        kg = nc.dram_tensor(f"kv_gath{qi}", [NC_, CKV], F16,
                            addr_space="Shared").ap()
        kv_locq.append((kl, kg))
        kt_locq.append(kl[:E * JJ].rearrange("(p l) -> p l", p=E))   # [512, 65]
        v_locq.append(kl[E * JJ:].rearrange("(l e) -> l e", e=E))    # [65, 512]
        ktgq.append(kg[:, :E * JJ].rearrange("r (p l) -> r p l", p=E))
        # gathered V of chunk c: [8 ranks, 65, 512]; row kappa = 65r + jj
        vgq.append(kg[:, E * JJ:].rearrange("r (l e) -> r l e", e=E))

    with tile.TileContext(nc) as tc:
        import contextlib
        ctx = contextlib.ExitStack()
        with ctx:
            pools = {
                "persist": ctx.enter_context(tc.tile_pool(name="persist", bufs=1)),
                "work": ctx.enter_context(tc.tile_pool(name="work", bufs=2)),
                "wts": ctx.enter_context(tc.tile_pool(name="wts", bufs=1)),
                "wts1": ctx.enter_context(tc.tile_pool(name="wts1", bufs=1)),
            }
            persist = pools["persist"]
            work = pools["work"]
            wts = pools["wts"]
            wts1 = pools["wts1"]

            # ------------- persistent constants -------------
            eps_t = persist.tile([128, 1], F32, tag="eps")
            nc.vector.memset(eps_t, 1e-5)
            vones_sb = []
            for p in range(5):
                lo, cw = CS[p]
                t = persist.tile([128, 8], F16, tag=f"vones{p}")
                nc.sync.dma_start(out=t[:cw], in_=vones_d[lo:lo + cw, :])
                vones_sb.append(t)
            rmask_sb = []
            for ti, (lo, cnt) in enumerate(RT):
                t = persist.tile([128, 1], F32, tag=f"rmask{ti}")
                nc.sync.dma_start(out=t[:cnt], in_=rmask_d[lo:lo + cnt, :])
                rmask_sb.append(t)
            # persistent V-chunk tiles (2 parity sets); ones cols written once
            vcs = []
            for par in range(2):
                row = []
                for p in range(5):
                    lo, cw = CS[p]
                    t = persist.tile([128, PC], F16, tag=f"vcs{par}_{p}",
                                     name=f"vcs{par}_{p}")
                    nc.vector.tensor_copy(
                        out=t[:cw].rearrange("p (h j) -> p h j", h=8)[:, :, 64:65],
                        in_=vones_sb[p][:cw].rearrange("p (h o) -> p h o", o=1))
                    row.append(t)
                vcs.append(row)

            def bcast(dram_vec, n, tag, pool=persist, parts=128):
                t = pool.tile([parts, n], F32, tag=tag)
                nc.sync.dma_start(out=t, in_=dram_vec.partition_broadcast(parts))
                return t

            # seq tiles (token-major f32), persistent across layers
            seq = [persist.tile([128 if i < 4 else 16, E], F32, tag=f"seq{i}", name=f"seq{i}")
                   for i in range(5)]
            nc.vector.memset(seq[4][:, :], 0.0)

            # ------------- front end: z projection + assembly -------------
            with tc.tile_pool(name="front", bufs=1) as fp, \
                 tc.tile_pool(name="frontp", bufs=2, space="PSUM") as fpp:
                pb_bc = bcast(pb_d, E, "pb_bc", pool=fp)
                zT_sb = []
                for cd in range(8):
                    t = fp.tile([128, 256], F16, tag=f"zT{cd}")
                    nc.sync.dma_start(out=t, in_=zT_d[128 * cd:128 * (cd + 1), :])
                    zT_sb.append(t)
                pw_sb = []
                for cd in range(8):
                    t = fp.tile([128, E], F16, tag=f"pw{cd}")
                    nc.sync.dma_start(out=t, in_=pw_d[128 * cd:128 * (cd + 1), :])
                    pw_sb.append(t)
                posz_sb = []
                for zt in range(2):
                    t = fp.tile([128, E], F32, tag=f"posz{zt}")
                    nc.sync.dma_start(out=t, in_=posz_d[128 * zt:128 * (zt + 1), :])
                    posz_sb.append(t)

                for zt in range(2):
                    ps = fpp.tile([128, E], F32, tag="psZ")
                    for cd in range(8):
                        nc.tensor.matmul(ps, zT_sb[cd][:, 128 * zt:128 * (zt + 1)],
                                         pw_sb[cd], start=(cd == 0), stop=(cd == 7))
                    zf = fp.tile([128, E], F32, tag="zf")
                    nc.vector.tensor_add(out=zf, in0=ps, in1=pb_bc)
                    # non-affine LN
                    stats = fp.tile([128, 6], F32, tag="zstats")
                    nc.vector.bn_stats(out=stats, in_=zf)
                    mv = fp.tile([128, 2], F32, tag="zmv")
                    nc.vector.bn_aggr(out=mv, in_=stats)
                    rs = fp.tile([128, 1], F32, tag="zrs")
                    nc.scalar.activation(out=rs, in_=mv[:, 1:2], func=AF.Sqrt,
                                         bias=eps_t, scale=1.0)
                    nc.vector.reciprocal(out=rs, in_=rs)
                    zn = fp.tile([128, E], F32, tag="zn")
                    nc.vector.tensor_scalar(out=zn, in0=zf,
                                            scalar1=mv[:, 0:1], scalar2=rs,
                                            op0=mybir.AluOpType.subtract,
                                            op1=mybir.AluOpType.mult)
                    # add pos (aligned), then scatter via DMA (any partition)
                    nc.vector.tensor_add(out=zn, in0=zn, in1=posz_sb[zt])
                    for cb in range(4):
                        c = 4 * zt + cb
                        for (t_i, po, cnt, off) in _row_spans(JJ * c, 32):
                            nc.sync.dma_start(
                                out=seq[t_i][po:po + cnt, :],
                                in_=zn[32 * cb + off:32 * cb + off + cnt, :])
                # rest rows -> seq
                for c in range(T):
                    for (t_i, po, cnt, off) in _row_spans(JJ * c + 32, 33):
                        nc.sync.dma_start(
                            out=seq[t_i][po:po + cnt, :],
                            in_=rest_d[33 * c + off:33 * c + off + cnt, :])

            # ------------- per-layer LN gain/bias bcast tiles -------------
            g1_bc = persist.tile([128, E], F32, tag="g1bc")
            b1_bc = persist.tile([128, E], F32, tag="b1bc")
            g2_bc = persist.tile([128, E], F32, tag="g2bc")
            b2_bc = persist.tile([128, E], F32, tag="b2bc")
            vb_bc = persist.tile([128, E], F32, tag="vbbc")
            ob_bc = persist.tile([128, E], F32, tag="obbc")
            mb2_bc = persist.tile([128, E], F32, tag="mb2bc")

            # ================= layers =================
            for li in range(DEPTH):
                for dst, src in ((g1_bc, ln1g_d[li]), (b1_bc, ln1b_d[li]),
                                 (g2_bc, ln2g_d[li]), (b2_bc, ln2b_d[li]),
                                 (vb_bc, vb_d[li]), (ob_bc, outb_d[li]),
                                 (mb2_bc, b2_d[li])):
                    nc.sync.dma_start(out=dst, in_=src.partition_broadcast(128))
                qkb_sb = wts.tile([128, 8], F32, tag="qkb")
                nc.sync.dma_start(out=qkb_sb, in_=qkbp_d[li])
                b1p_sb = wts.tile([128, 16], F32, tag="b1p")
                nc.sync.dma_start(out=b1p_sb, in_=b1p_d[li])

                qkw_sb = []
                for e in range(4):
                    t = wts.tile([128, 2 * E], F16, tag=f"qkw{e}")
                    nc.sync.dma_start(out=t, in_=qkw_d[li, 128 * e:128 * (e + 1), :])
                    qkw_sb.append(t)
                vw_sb = []
                for e in range(4):
                    t = wts.tile([128, E], F16, tag=f"vw{e}")
                    nc.sync.dma_start(out=t, in_=vw_d[li, 128 * e:128 * (e + 1), :])
                    vw_sb.append(t)
                ow_sb = []
                for q in range(4):
                    t = wts.tile([128, E], F16, tag=f"ow{q}")
                    nc.sync.dma_start(out=t, in_=ow_d[li, 128 * q:128 * (q + 1), :])
                    ow_sb.append(t)
                w1_sb = []
                for e in range(4):
                    t = wts1.tile([128, 4 * E], F16, tag=f"w1_{e}")
                    nc.sync.dma_start(out=t, in_=w1_d[li, 128 * e:128 * (e + 1), :])
                    w1_sb.append(t)
                w2_sb = []
                for cd in range(16):
                    t = wts1.tile([128, E], F16, tag=f"w2_{cd}")
                    nc.sync.dma_start(out=t, in_=w2_d[li, 128 * cd:128 * (cd + 1), :])
                    w2_sb.append(t)

                # ---- LN1 + transpose ----
                h1 = _ln_layer(nc, pools, seq, g1_bc, b1_bc, eps_t, F16)
                h1T = _transpose_rows(nc, pools, h1, "hT")

                # ---- qkv^T (Q,K) ----
                qkT = [work.tile([128, PC], F16, tag=f"qkT{m}", name=f"qkT{m}", bufs=1)
                       for m in range(4)]
                qkTK = work.tile([128, 4, PC], F16, tag="qkTK", name="qkTK",
                                 bufs=1)
                def _qk_tiles(ms):
                    for m in ms:
                        for (no, nw) in _nsplits(PC):
                            ps = pB.tile([128, 512], F32, tag="psqk")
                            for e in range(4):
                                nc.tensor.matmul(
                                    ps[:, :nw],
                                    qkw_sb[e][:, 128 * m:128 * (m + 1)],
                                    h1T[e][:, no:no + nw],
                                    start=(e == 0), stop=(e == 3))
                            dst = (qkT[m][:, no:no + nw] if m < 4 else
                                   qkTK[:, m - 4, no:no + nw])
                            nc.vector.tensor_scalar_add(
                                out=dst, in0=ps[:, :nw],
                                scalar1=qkb_sb[:, m:m + 1])

                with tc.tile_pool(name="pB", bufs=3, space="PSUM") as pB:
                    _qk_tiles([4, 5, 6, 7])        # K^T first
                    # ---- V natural (single tensor [128, 5, 512]) ----
                    v_all = work.tile([128, 5, E], F16, tag="vall", name="vall",
                                      bufs=1)
                    for ti, (lo, cnt) in enumerate(RT):
                        real = RT_REAL[ti]
                        ps = pB.tile([128, 512], F32, tag="psv")
                        for e in range(4):
                            nc.tensor.matmul(ps[:real],
                                             h1T[e][:, lo:lo + real],
                                             vw_sb[e],
                                             start=(e == 0), stop=(e == 3))
                        nc.vector.tensor_add(out=v_all[:real, ti, :],
                                             in0=ps[:real], in1=vb_bc[:real])
                        nc.vector.tensor_scalar_mul(
                            out=v_all[:real, ti, :], in0=v_all[:real, ti, :],
                            scalar1=rmask_sb[ti][:real])

                    # ---- ship K^T/V per chunk; AGs pipeline with attention ----
                    def _ship_chunk(qi):
                        qs = JJ * qi
                        kl, kg = kv_locq[qi]
                        nc.sync.dma_start(
                            out=kt_locq[qi].rearrange("(g p) j -> p g j", g=4),
                            in_=qkTK[:, :, qs:qs + JJ])
                        for (t_i, po, cnt, off) in _row_spans(qs, JJ):
                            rc = min(cnt, max(0, RT_REAL[t_i] - po))
                            if rc > 0:
                                nc.scalar.dma_start(
                                    out=v_locq[qi][off:off + rc, :],
                                    in_=v_all[po:po + rc, t_i, :])
                        nc.gpsimd.collective_compute(
                            "AllGather", mybir.AluOpType.bypass,
                            replica_groups=[list(range(NC_))],
                            ins=[kl[:]], outs=[kg[:]])

                    for qi in range(T):
                        _ship_chunk(qi)
                    _qk_tiles([0, 1, 2, 3])        # Q^T overlaps AGs

                # ---- attention ----
                oT = [work.tile([65, 2, PC], F32, tag=f"oT{pr}", name=f"oT{pr}",
                                bufs=1) for pr in range(4)]
                with tc.tile_pool(name="pS", bufs=2, space="PSUM") as pS, \
                     tc.tile_pool(name="pO", bufs=2, space="PSUM") as pO:
                    for cpr in range(T):
                        # gather K^T chunk (4 tiles) and V chunk (5 tiles)
                        ktc = []
                        for q in range(4):
                            t = work.tile([128, PC], F16, tag=f"ktc{q}", bufs=2)
                            nc.sync.dma_start(
                                out=t.rearrange("p (k j) -> p k j", k=8),
                                in_=ktgq[cpr][:, 128 * q:128 * (q + 1), :]
                                .rearrange("r p j -> p r j"))
                            ktc.append(t)
                        vc = vcs[cpr % 2]
                        pieces = 0
                        for p in range(5):
                            lo, cw = CS[p]
                            done = 0
                            while done < cw:
                                g = lo + done
                                r = g // JJ
                                j0 = g % JJ
                                cnt = min(JJ - j0, cw - done)
                                eng = nc.sync if pieces % 2 else nc.gpsimd
                                eng.dma_start(
                                    out=vc[p][done:done + cnt]
                                    .rearrange("p (h j) -> p h j", h=8)[:, :, 0:64],
                                    in_=vgq[cpr][r, j0:j0 + cnt, :]
                                    .rearrange("l (h d) -> l h d", h=8))
                                done += cnt
                                pieces += 1

                        row_lo = JJ * cpr
                        passes = ([(0, 260), (260, 260)] if cpr == 0
                                  else [(row_lo, LR - row_lo)])
                        for (rlo, rcnt) in passes:
                            for pr in range(4):
                                poT = pO.tile([65, 2, 512], F32, tag="oTp")
                                for s in range(5):
                                    clo, cw = CS[s]
                                    psq = pS.tile([128, 2, 512], F32, tag="Sp")
                                    for hh in range(2):
                                        h = 2 * pr + hh
                                        nc.tensor.matmul(
                                            psq[:cw, hh, :rcnt],
                                            ktc[h // 2][64 * (h % 2):64 * (h % 2) + 64,
                                                        clo:clo + cw],
                                            qkT[h // 2][64 * (h % 2):64 * (h % 2) + 64,
                                                        rlo:rlo + rcnt],
                                            start=True, stop=True)
                                    eq = work.tile([128, 2, PC], F16,
                                                   tag="Eq", bufs=3)
                                    nc.scalar.activation(
                                        out=eq[:cw, :, :rcnt],
                                        in_=psq[:cw, :, :rcnt],
                                        func=AF.Exp, scale=float(SCALE))
                                    for hh in range(2):
                                        h = 2 * pr + hh
                                        nc.tensor.matmul(
                                            poT[:, hh, :rcnt],
                                            vc[s][:cw, JJ * h:JJ * h + JJ],
                                            eq[:cw, hh, :rcnt],
                                            start=(s == 0), stop=(s == 4))
                                if cpr == 0:
                                    nc.vector.tensor_copy(
                                        out=oT[pr][:, :, rlo:rlo + rcnt],
                                        in_=poT[:, :, :rcnt])
                                else:
                                    nc.vector.tensor_add(
                                        out=oT[pr][:, :, rlo:rlo + rcnt],
                                        in0=oT[pr][:, :, rlo:rlo + rcnt],
                                        in1=poT[:, :, :rcnt])

                # ---- normalize + out projection + residual ----
                oTn = [work.tile([128, PC], F16, tag=f"oTn{q}", name=f"oTn{q}", bufs=1)
                       for q in range(4)]
                for h in range(H):
                    nc.sync.dma_start(out=dn_dram[h:h + 1, :],
                                      in_=oT[h // 2][64:65, h % 2, :])
                dn8 = work.tile([8, PC], F32, tag="dn8", bufs=1)
                nc.sync.dma_start(out=dn8, in_=dn_dram[:, :])
                nc.vector.reciprocal(out=dn8, in_=dn8)
                nc.sync.dma_start(out=dn_dram[:, :], in_=dn8)
                for h in range(H):
                    rb = work.tile([64, PC], F32, tag="rcpb", bufs=2)
                    nc.sync.dma_start(
                        out=rb, in_=dn_dram[h, :].partition_broadcast(64))
                    nc.vector.tensor_mul(
                        out=oTn[h // 2][64 * (h % 2):64 * (h % 2) + 64, :],
                        in0=oT[h // 2][0:64, h % 2, :], in1=rb)
                with tc.tile_pool(name="pF", bufs=2, space="PSUM") as pF:
                    for ti, (lo, cnt) in enumerate(RT):
                        real = RT_REAL[ti]
                        ps = pF.tile([128, 512], F32, tag="psF")
                        for q in range(4):
                            nc.tensor.matmul(ps[:real],
                                             oTn[q][:, lo:lo + real],
                                             ow_sb[q],
                                             start=(q == 0), stop=(q == 3))
                        nc.vector.tensor_add(out=ps[:real], in0=ps[:real],
                                             in1=ob_bc[:real])
                        nc.vector.tensor_add(out=seq[ti][:real],
                                             in0=seq[ti][:real], in1=ps[:real])

                # ---- LN2 + MLP ----
                h2 = _ln_layer(nc, pools, seq, g2_bc, b2_bc, eps_t, F16)
                h2T = _transpose_rows(nc, pools, h2, "hT")
                with tc.tile_pool(name="pG", bufs=3, space="PSUM") as pG:
                    for (rlo, rcnt) in ((0, 256), (256, 264)):
                        gT = [work.tile([128, 264], F16, tag=f"gT{m}", name=f"gT{m}", bufs=1)
                              for m in range(16)]
                        for m in range(16):
                            ps = pG.tile([128, 512], F32, tag="psG")
                            for e in range(4):
                                nc.tensor.matmul(
                                    ps[:, :rcnt],
                                    w1_sb[e][:, 128 * m:128 * (m + 1)],
                                    h2T[e][:, rlo:rlo + rcnt],
                                    start=(e == 0), stop=(e == 3))
                            nc.scalar.activation(out=gT[m][:, :rcnt],
                                                 in_=ps[:, :rcnt],
                                                 func=AF.Gelu_apprx_tanh,
                                                 bias=b1p_sb[:, m:m + 1],
                                                 scale=1.0)
                        for (t_i, po, cnt, off) in _row_spans(rlo, rcnt):
                            real = min(cnt, max(0, RT_REAL[t_i] - po))
                            if real <= 0:
                                continue
                            ps = pG.tile([128, 512], F32, tag="psM")
                            for cd in range(16):
                                nc.tensor.matmul(
                                    ps[:real],
                                    gT[cd][:, off:off + real],
                                    w2_sb[cd],
                                    start=(cd == 0), stop=(cd == 15))
                            nc.vector.tensor_add(out=ps[:real], in0=ps[:real],
                                                 in1=mb2_bc[:real])
                            nc.vector.tensor_add(out=seq[t_i][po:po + real],
                                                 in0=seq[t_i][po:po + real],
                                                 in1=ps[:real])

            # ================= output =================
            with tc.tile_pool(name="tail", bufs=1) as tp, \
                 tc.tile_pool(name="tailp", bufs=2, space="PSUM") as tpp:
                ng_bc = bcast(ng_d, E, "ngbc", pool=tp)
                nb_bc = bcast(nb_d, E, "nbbc", pool=tp)
                hf = _ln_layer(nc, pools, seq, ng_bc, nb_bc, eps_t, F16)
                hfT = _transpose_rows(nc, pools, hf, "hT")  # reuse slots
                hq = [tp.tile([128, 264], F16, tag=f"hq{e}", name=f"hq{e}") for e in range(4)]
                for e in range(4):
                    for c in range(T):
                        nc.sync.dma_start(
                            out=hq[e][:, 33 * c:33 * (c + 1)],
                            in_=hfT[e][:, JJ * c + 32:JJ * c + JJ])
                opw_sb = []
                for e in range(4):
                    t = tp.tile([128, D], F16, tag=f"opw{e}")
                    nc.sync.dma_start(out=t, in_=opw_d[128 * e:128 * (e + 1), :])
                    opw_sb.append(t)
                opb_bc = tp.tile([128, D], F32, tag="opbbc")
                nc.sync.dma_start(out=opb_bc, in_=opb_d.partition_broadcast(128))

                for (mlo, mcnt) in ((0, 128), (128, 128), (256, 8)):
                    ot = tp.tile([128, D], F32, tag="otile")
                    for nn in range(2):
                        ps = tpp.tile([128, 512], F32, tag="psO")
                        for e in range(4):
                            nc.tensor.matmul(
                                ps[:mcnt],
                                hq[e][:, mlo:mlo + mcnt],
                                opw_sb[e][:, 512 * nn:512 * (nn + 1)],
                                start=(e == 0), stop=(e == 3))
                        nc.vector.tensor_add(
                            out=ot[:mcnt, 512 * nn:512 * (nn + 1)],
                            in0=ps[:mcnt],
                            in1=opb_bc[:mcnt, 512 * nn:512 * (nn + 1)])
                    nc.sync.dma_start(out=out_d[mlo:mlo + mcnt, :],
                                      in_=ot[:mcnt])

    nc.compile()
    nc.m = get_hw_module(nc.m)
    return nc


# ---------------- host side ----------------

def _ln_np(x, eps=1e-5):
    m = x.mean(-1, keepdims=True)
    v = ((x - m) ** 2).mean(-1, keepdims=True)
    return (x - m) / np.sqrt(v + eps)


def make_in_maps(inputs):
    f = {n: np.asarray(v) for n, v in inputs.items()}
    z = f["z_past"][0]                   # [8, 256, 1024]
    code = f["code_embeddings"][0]       # [8, 3, 128]
    q = f["query_embed"][0]              # [256, 512]
    pos = f["pos_embed"][0]              # [10300, 512]

    shared = {
        "vones": np.repeat(
            np.array([[1.0 if (8 * jj + kk) < CHUNK else 0.0
                       for jj in range(JJ)] for kk in range(8)],
                     np.float16).reshape(PC, 1), 8, axis=1).astype(np.float16),
        "patch_w": f["patch_w"].astype(np.float16),
        "patch_b": f["patch_b"].astype(np.float32),
        "qk_w": f["qkv_w"][:, :, :2 * E].astype(np.float16),
        "v_w": f["qkv_w"][:, :, 2 * E:].astype(np.float16),
        "o_w": f["out_w"].astype(np.float16),
        "w1": f["mlp_w1"].astype(np.float16),
        "w2": f["mlp_w2"].astype(np.float16),
        "qkb_p": np.ascontiguousarray(
            f["qkv_b"][:, :2 * E].reshape(DEPTH, 8, 128).transpose(0, 2, 1)
        ).astype(np.float32),
        "v_b": f["qkv_b"][:, 2 * E:].astype(np.float32),
        "out_b": f["out_b"].astype(np.float32),
        "b1_p": np.ascontiguousarray(
            f["mlp_b1"].reshape(DEPTH, 16, 128).transpose(0, 2, 1)
        ).astype(np.float32),
        "b2": f["mlp_b2"].astype(np.float32),
        "ln1g": f["ln1_g"].astype(np.float32),
        "ln1b": f["ln1_b"].astype(np.float32),
        "ln2g": f["ln2_g"].astype(np.float32),
        "ln2b": f["ln2_b"].astype(np.float32),
        "norm_g": f["norm_g"].astype(np.float32),
        "norm_b": f["norm_b"].astype(np.float32),
        "oproj_w": f["oproj_w"].astype(np.float16),
        "oproj_b": f["oproj_b"].astype(np.float32),
    }
    # vones: row kappa = 65k + jj -> built with kk outer above; fix order
    v1 = np.zeros((PC, 8), np.float16)
    for kk in range(8):
        for jj in range(JJ):
            v1[JJ * kk + jj, :] = 1.0 if (8 * jj + kk) < CHUNK else 0.0
    shared["vones"] = v1

    c_proj = _ln_np(code.astype(np.float32) @ f["code_w"] + f["code_b"])

    in_maps = []
    for k in range(NC_):
        zk = z[:, k::8, :].reshape(256, D)
        zT = np.ascontiguousarray(zk.T).astype(np.float16)
        posz = np.zeros((256, E), np.float32)
        rest = np.zeros((T * 33, E), np.float32)
        for c in range(T):
            for jj in range(32):
                posz[32 * c + jj] = pos[515 * c + 8 * jj + k]
            for jj in range(32, JJ):
                p = 8 * jj + k
                ri = 33 * c + (jj - 32)
                if p < 259:
                    rest[ri] = c_proj[c, p - 256] + pos[515 * c + p]
                elif p < CHUNK:
                    rest[ri] = q[p - 259] + pos[515 * c + p]
        rowmask = np.array(
            [[1.0 if (l < LR and (8 * (l % JJ) + k) < CHUNK) else 0.0]
             for l in range(LRP)], np.float32)
        m = dict(shared)
        m["zT"] = zT
        m["posz"] = posz
        m["rest"] = rest
        m["rowmask"] = rowmask
        in_maps.append(m)
    return in_maps


def unshard_output(results, dtype):
    out = np.zeros((1, T, N, D), np.float32)
    for k in range(NC_):
        pred = results[k]["out"]          # [264, 1024]
        for c in range(T):
            for i2 in range(33):
                p = 256 + 8 * i2 + k
                if 259 <= p < CHUNK:
                    out[0, c, p - 259] = pred[33 * c + i2]
    return out.astype(dtype)


_PROG_LOCK = threading.Lock()
_PROG = None


def _get_prog():
    global _PROG
    with _PROG_LOCK:
        if _PROG is None:
            _PROG = build_program()
    return _PROG


def kernel(**inputs):
    nc = _get_prog()
    in_maps = make_in_maps(inputs)
    res = run_bass_kernel_spmd(nc, in_maps, list(range(NC_)))
    return unshard_output(res.results, np.asarray(inputs["z_past"]).dtype)


if __name__ == "__main__":
    nc = build_program()
    print("program built ok")



# revision 35
# speedup vs baseline: 1.1081x; 1.1081x over previous
"""Trainium2 Bass kernel for nn_BatchedVQLAMDecoder (8-core SPMD).

Sharding: mod-8 interleave of the 4120-token sequence (8 chunks of 515,
padded to 520). Core k owns padded rows p == k (mod 8) of every chunk.
Block-causal mask at chunk granularity => uniform SPMD.

v3 (all fp16 matmuls):
- 4 AllGathers per layer (2 chunks each) instead of 8 small ones.
- V shipped in head-pair layout [V_even|ones|V_odd]x4 (768 wide) so the
  gather is 3 contiguous DMAs and the softmax denominators come out of
  the same AV matmul (ones block), aligned for the normalize multiply.
- col kappa = 64r + l (l<64), extras l=64 at 512+r: contiguous gathers.
- merged weight/bias DMAs; oT accumulation split across DVE and Pool.
"""

import threading

import numpy as np

import bass_rust
import concourse.bass as bass
import concourse.tile as tile
from concourse import bacc, mybir
from concourse.bass_utils import run_bass_kernel_spmd
from concourse.bass_interp import get_hw_module

F32 = mybir.dt.float32
F16 = mybir.dt.float16
U8 = mybir.dt.uint8
AF = mybir.ActivationFunctionType

T, N, D, CDim, E, H, DEPTH = 8, 256, 1024, 128, 512, 8, 3
Dh = E // H                   # 64
CHUNK = 2 * N + 3             # 515
JJ = 65                       # local rows per (core, chunk)
PC = 8 * JJ                   # padded chunk = 520
LR = T * JJ                   # local rows per core = 520
LRP = 528                     # padded to mult-16 for dma transpose
NC_ = 8
SCALE = 1.0 / np.sqrt(Dh)

KBYTES = E * JJ * 2           # fp16 K^T slab bytes per chunk = 66560
VW = 768                      # V slab width: 4x [V_even(64)|ones(64)|V_odd(64)]
VBYTES = JJ * VW * 2          # fp16 V slab bytes per chunk = 99840
CKVB = KBYTES + VBYTES        # 166400
PAIRB = 2 * CKVB              # AG payload: 2 chunks

RT_REAL = [128, 128, 128, 128, 8]
CS = [(0, 128), (128, 128), (256, 128), (384, 128), (512, 8)]


def _row_spans(lo, cnt):
    out = []
    done = 0
    while done < cnt:
        g = lo + done
        t = g // 128
        po = g % 128
        c = min(128 - po, cnt - done)
        out.append((t, po, c, done))
        done += c
    return out


def _mk_ap(x, dims):
    c = x.copy()
    c.ap = bass_rust.VecI64Pair([[int(a), int(b)] for a, b in dims])
    return c


def _pstride(x):
    return int([list(p) for p in x.ap][0][0])


def build_program():
    nc = bacc.Bacc("TRN2", target_bir_lowering=False, debug=False,
                   num_devices=NC_)

    def inp(name, shape, dt):
        return nc.dram_tensor(name, list(shape), dt, kind="ExternalInput").ap()

    zT_d = inp("zT", [D, 2 * CDim], F16)
    posz_d = inp("posz", [2 * CDim, E], F32)
    rest_d = inp("rest", [T * 33, E], F32)
    rmask_d = inp("rowmask", [640], F32)
    pw_d = inp("patch_w", [D, E], F16)
    pb_d = inp("patch_b", [E], F32)
    qkw_d = inp("qkw16", [DEPTH, 128, 4, 2 * E], F16)
    vw_d = inp("vw16", [DEPTH, 128, 4, E], F16)
    ow_d = inp("ow16", [DEPTH, 128, 4, E], F16)
    w1_d = inp("w1_16", [DEPTH, 128, 4, 4 * E], F16)
    w2_d = inp("w2_16", [DEPTH, 128, 16, E], F16)
    pb24_d = inp("pb24", [DEPTH, 128, 24], F32)
    lnv_d = inp("lnvec", [DEPTH, 7, E], F32)
    ng_d = inp("norm_g", [E], F32)
    nb_d = inp("norm_b", [E], F32)
    opw_d = inp("oproj_w", [E, D], F16)
    opb_d = inp("oproj_b", [D], F32)

    out_d = nc.dram_tensor("out", [T * 33, D], F32, kind="ExternalOutput").ap()

    kv_loc = [nc.dram_tensor(f"kv_loc{h}", [PAIRB], U8).ap() for h in range(4)]
    kv_g = [nc.dram_tensor(f"kv_g{h}", [NC_, PAIRB], U8,
                           addr_space="Shared").ap() for h in range(4)]

    with tile.TileContext(nc) as tc:
        import contextlib
        ctx = contextlib.ExitStack()
        with ctx:
            persist = ctx.enter_context(tc.tile_pool(name="persist", bufs=1))
            work = ctx.enter_context(tc.tile_pool(name="work", bufs=2))
            wts = ctx.enter_context(tc.tile_pool(name="wts", bufs=1))

            eps_t = persist.tile([128, 1], F32, tag="eps")
            nc.vector.memset(eps_t, 1e-5)
            rmask_sb = persist.tile([128, 5], F32, tag="rmask")
            nc.sync.dma_start(
                out=rmask_sb, in_=rmask_d.rearrange("(g p) -> p g", p=128))
            # V chunk tiles (2 parity): per head pair pr:
            # [V(2pr) 64 | ones 64 | V(2pr+1) 64]
            vc = [persist.tile([128, 5, VW], F16, tag=f"vc{par}",
                               name=f"vc{par}") for par in range(2)]
            seq = persist.tile([128, 5, E], F32, tag="seq", name="seq")
            nc.vector.memset(seq, 0.0)

            # ------------- front end -------------
            with tc.tile_pool(name="front", bufs=1) as fp, \
                 tc.tile_pool(name="frontp", bufs=2, space="PSUM") as fpp:
                pb_bc = fp.tile([128, E], F32, tag="pb_bc")
                nc.sync.dma_start(out=pb_bc, in_=pb_d.partition_broadcast(128))
                zT_sb = fp.tile([128, 8, 256], F16, tag="zT")
                nc.sync.dma_start(
                    out=zT_sb, in_=zT_d.rearrange("(c p) n -> p c n", p=128))
                pw_sb = fp.tile([128, 8, E], F16, tag="pw")
                nc.sync.dma_start(
                    out=pw_sb, in_=pw_d.rearrange("(c p) n -> p c n", p=128))
                posz_sb = fp.tile([128, 2, E], F32, tag="posz")
                nc.sync.dma_start(
                    out=posz_sb, in_=posz_d.rearrange("(t p) e -> p t e", p=128))

                for zt in range(2):
                    ps = fpp.tile([128, E], F32, tag="psZ")
                    for cd in range(8):
                        nc.tensor.matmul(ps, zT_sb[:, cd, 128 * zt:128 * (zt + 1)],
                                         pw_sb[:, cd, :],
                                         start=(cd == 0), stop=(cd == 7))
                    zf = fp.tile([128, E], F32, tag="zf")
                    nc.vector.tensor_add(out=zf, in0=ps, in1=pb_bc)
                    stats = fp.tile([128, 6], F32, tag="zstats")
                    nc.vector.bn_stats(out=stats, in_=zf)
                    mv = fp.tile([128, 2], F32, tag="zmv")
                    nc.vector.bn_aggr(out=mv, in_=stats)
                    rs = fp.tile([128, 1], F32, tag="zrs")
                    nc.scalar.activation(out=rs, in_=mv[:, 1:2], func=AF.Sqrt,
                                         bias=eps_t, scale=1.0)
                    nc.vector.reciprocal(out=rs, in_=rs)
                    zn = fp.tile([128, E], F32, tag="zn")
                    nc.vector.tensor_scalar(out=zn, in0=zf,
                                            scalar1=mv[:, 0:1], scalar2=rs,
                                            op0=mybir.AluOpType.subtract,
                                            op1=mybir.AluOpType.mult)
                    nc.vector.tensor_add(out=zn, in0=zn, in1=posz_sb[:, zt, :])
                    for cb in range(4):
                        c = 4 * zt + cb
                        for (g, po, cnt, off) in _row_spans(JJ * c, 32):
                            nc.sync.dma_start(
                                out=seq[po:po + cnt, g, :],
                                in_=zn[32 * cb + off:32 * cb + off + cnt, :])
                for c in range(T):
                    for (g, po, cnt, off) in _row_spans(JJ * c + 32, 33):
                        nc.sync.dma_start(
                            out=seq[po:po + cnt, g, :],
                            in_=rest_d[33 * c + off:33 * c + off + cnt, :])

            # ================= layers =================
            for li in range(DEPTH):
                lnv_sb = wts.tile([128, 7, E], F32, tag="lnv")
                nc.sync.dma_start(
                    out=lnv_sb,
                    in_=lnv_d[li].rearrange("v e -> (v e)")
                    .partition_broadcast(128).rearrange("p (v e) -> p v e", v=7))
                pb24_sb = wts.tile([128, 24], F32, tag="pb24")
                nc.sync.dma_start(out=pb24_sb, in_=pb24_d[li])
                qkw_sb = wts.tile([128, 4, 2 * E], F16, tag="qkw", bufs=2)
                nc.sync.dma_start(out=qkw_sb, in_=qkw_d[li])
                vw_sb = wts.tile([128, 4, E], F16, tag="vw")
                nc.sync.dma_start(out=vw_sb, in_=vw_d[li])
                ow_sb = wts.tile([128, 4, E], F16, tag="ow")
                nc.sync.dma_start(out=ow_sb, in_=ow_d[li])
                w1_sb = wts.tile([128, 4, 4 * E], F16, tag="w1")
                nc.sync.dma_start(out=w1_sb, in_=w1_d[li])
                w2_sb = wts.tile([128, 16, E], F16, tag="w2")
                nc.sync.dma_start(out=w2_sb, in_=w2_d[li])

                def _ln(gi, bi, tag):
                    h16 = work.tile([128, 5, E], F16, tag=tag, name=tag,
                                    bufs=1)
                    for g in range(5):
                        cnt = 16 if g == 4 else 128
                        stats = work.tile([128, 6], F32, tag="ln_st")
                        nc.vector.bn_stats(out=stats[:cnt], in_=seq[:cnt, g, :])
                        mv = work.tile([128, 2], F32, tag="ln_mv")
                        nc.vector.bn_aggr(out=mv[:cnt], in_=stats[:cnt])
                        rs = work.tile([128, 1], F32, tag="ln_rs")
                        nc.scalar.activation(out=rs[:cnt], in_=mv[:cnt, 1:2],
                                             func=AF.Sqrt, bias=eps_t[:cnt],
                                             scale=1.0)
                        nc.vector.reciprocal(out=rs[:cnt], in_=rs[:cnt])
                        y = work.tile([128, E], F32, tag="ln_y")
                        nc.vector.tensor_scalar(out=y[:cnt], in0=seq[:cnt, g, :],
                                                scalar1=mv[:cnt, 0:1],
                                                scalar2=rs[:cnt],
                                                op0=mybir.AluOpType.subtract,
                                                op1=mybir.AluOpType.mult)
                        nc.vector.tensor_mul(out=y[:cnt], in0=y[:cnt],
                                             in1=lnv_sb[:cnt, gi, :])
                        nc.vector.tensor_add(out=h16[:cnt, g, :], in0=y[:cnt],
                                             in1=lnv_sb[:cnt, bi, :])
                    return h16

                def _transpose(h16, tag):
                    hT = work.tile([128, 4, LRP], F16, tag=tag, name=tag, bufs=1)
                    for g in range(5):
                        pcnt = 16 if g == 4 else 128
                        for e in range(4):
                            nc.scalar.dma_start_transpose(
                                out=hT[:, e, 128 * g:128 * g + pcnt],
                                in_=h16[0:pcnt, g, 128 * e:128 * (e + 1)])
                    return hT

                h1T = _transpose(_ln(0, 1, "h1"), "hT")

                qkQ = work.tile([128, 4, LRP], F16, tag="qkQ", name="qkQ",
                                bufs=1)
                qkK = work.tile([128, 4, LRP], F16, tag="qkK", name="qkK",
                                bufs=1)

                def _qk_tiles(pB, ms):
                    for m in ms:
                        for (no, nw) in ((0, 512), (512, 8)):
                            ps = pB.tile([128, 512], F32, tag="psqk")
                            for e in range(4):
                                nc.tensor.matmul(
                                    ps[:, :nw],
                                    qkw_sb[:, e, 128 * m:128 * (m + 1)],
                                    h1T[:, e, no:no + nw],
                                    start=(e == 0), stop=(e == 3))
                            dst = (qkQ[:, m, no:no + nw] if m < 4 else
                                   qkK[:, m - 4, no:no + nw])
                            nc.vector.tensor_scalar_add(
                                out=dst, in0=ps[:, :nw],
                                scalar1=pb24_sb[:, m:m + 1])

                with tc.tile_pool(name="pB", bufs=3, space="PSUM") as pB:
                    _qk_tiles(pB, [4, 5, 6, 7])      # K^T first
                    # V natural [row%128, g, VW] f16, head-pair layout
                    v16 = work.tile([128, 5, VW], F16, tag="v16", name="v16",
                                    bufs=1)
                    for g in range(5):
                        real = RT_REAL[g]
                        ps = pB.tile([128, 512], F32, tag="psv")
                        for e in range(4):
                            nc.tensor.matmul(
                                ps[:real],
                                h1T[:, e, 128 * g:128 * g + real],
                                vw_sb[:, e, :],
                                start=(e == 0), stop=(e == 3))

                        def _s4(x, st, real=real):
                            return _mk_ap(x, [[_pstride(x), real],
                                              [st, 4], [1, 64]])
                        nc.vector.memset(_s4(v16[:real, g, 64:128], 192), 1.0)
                        for par in range(2):
                            nc.vector.tensor_add(
                                out=_s4(v16[:real, g, 128 * par:
                                            128 * par + 64], 192),
                                in0=_s4(ps[:real, 64 * par:64 * par + 64], 128),
                                in1=_s4(lnv_sb[:real, 4, 64 * par:
                                               64 * par + 64], 128))
                        nc.vector.tensor_scalar_mul(
                            out=v16[:real, g, :], in0=v16[:real, g, :],
                            scalar1=rmask_sb[:real, g:g + 1])

                    # ship K^T/V per chunk; 4 AGs per layer (2 chunks each)
                    for c in range(T):
                        grp, rel = c // 2, c % 2
                        base = rel * CKVB
                        kdst = (kv_loc[grp][base:base + KBYTES]
                                .bitcast(F16)
                                .rearrange("(g p j) -> p g j", g=4, p=128))
                        nc.sync.dma_start(out=kdst,
                                          in_=qkK[:, :, JJ * c:JJ * c + JJ])
                        vdst = (kv_loc[grp][base + KBYTES:base + CKVB]
                                .bitcast(F16)
                                .rearrange("(l e) -> l e", e=VW))
                        for (g, po, cnt, off) in _row_spans(JJ * c, JJ):
                            nc.sync.dma_start(
                                out=vdst[off:off + cnt, :],
                                in_=v16[po:po + cnt, g, :])
                        if rel == 1:
                            nc.gpsimd.collective_compute(
                                "AllGather", mybir.AluOpType.bypass,
                                replica_groups=[list(range(NC_))],
                                ins=[kv_loc[grp][:]], outs=[kv_g[grp][:, :]])
                    _qk_tiles(pB, [0, 1, 2, 3])      # Q^T overlaps AGs

                # ---- attention ----
                oT = [work.tile([128, 2, PC], F32, tag=f"oT{pr}",
                                name=f"oT{pr}", bufs=1) for pr in range(4)]
                with tc.tile_pool(name="pS", bufs=2, space="PSUM") as pS, \
                     tc.tile_pool(name="pO", bufs=2, space="PSUM") as pO:
                    for cpr in range(T):
                        grp, rel = cpr // 2, cpr % 2
                        base = rel * CKVB
                        ktc = work.tile([128, 4, PC], F16, tag="ktc", bufs=2)
                        ksrc = (kv_g[grp][:, base:base + KBYTES]
                                .bitcast(F16)
                                .rearrange("r (g p j) -> p g r j", g=4, p=128))
                        for qb in range(4):
                            nc.sync.dma_start(
                                out=ktc[:, qb, 0:512],
                                in_=ksrc[:, qb:qb + 1, :, 0:64].squeeze(1))
                            nc.sync.dma_start(
                                out=ktc[:, qb, 512:520],
                                in_=ksrc[:, qb:qb + 1, :, 64:65]
                                .squeeze(3).squeeze(1))
                        vcp = vc[cpr % 2]
                        vsrc = (kv_g[grp][:, base + KBYTES:base + CKVB]
                                .bitcast(F16)
                                .rearrange("r (l e) -> r l e", e=VW))
                        # col kappa = 64r + l (l<64); extras l=64 at 512+r
                        for par0 in range(2):
                            dst0 = vcp[64 * par0:64 * par0 + 64, 0, :]
                            d3 = _mk_ap(dst0, [[_pstride(dst0), 64],
                                               [VW, 4], [1, VW]])
                            nc.sync.dma_start(
                                out=d3,
                                in_=vsrc[par0:8:2, 0:64, :]
                                .rearrange("r l e -> l r e"))
                        nc.sync.dma_start(out=vcp[0:8, 4, :],
                                          in_=vsrc[:, 64:65, :].squeeze(1))

                        passes = ([(0, 512), (512, 8)] if cpr == 0
                                  else [(JJ * cpr, LR - JJ * cpr)])
                        for (rlo, rcnt) in passes:
                            for pr in range(4):
                                poT = pO.tile([128, 2, 512], F32, tag="poT")
                                for s in range(5):
                                    lo, cw = CS[s]
                                    psq = pS.tile([128, 2, 512], F32, tag="psq")
                                    for hh in range(2):
                                        nc.tensor.matmul(
                                            psq[:cw, hh, :rcnt],
                                            ktc[64 * hh:64 * hh + 64,
                                                pr, lo:lo + cw],
                                            qkQ[64 * hh:64 * hh + 64,
                                                pr, rlo:rlo + rcnt],
                                            start=True, stop=True)
                                    eq = work.tile([128, 2, PC], F16,
                                                   tag="eq", bufs=2)
                                    nc.scalar.activation(
                                        out=eq[:cw, :, :rcnt],
                                        in_=psq[:cw, :, :rcnt],
                                        func=AF.Exp, scale=float(SCALE))
                                    for hh in range(2):
                                        off = 192 * pr + 64 * hh
                                        nc.tensor.matmul(
                                            poT[:, hh, :rcnt],
                                            vcp[0:cw, s, off:off + 128],
                                            eq[:cw, hh, :rcnt],
                                            start=(s == 0), stop=(s == 4))
                                eng = nc.vector
                                if cpr == 0:
                                    eng.tensor_copy(
                                        out=oT[pr][:, :, rlo:rlo + rcnt],
                                        in_=poT[:, :, :rcnt])
                                else:
                                    eng.tensor_add(
                                        out=oT[pr][:, :, rlo:rlo + rcnt],
                                        in0=oT[pr][:, :, rlo:rlo + rcnt],
                                        in1=poT[:, :, :rcnt])

                # ---- normalize (denominators already in oT) ----
                # slot0: o(2pr) @p0..64, den(2pr) @p64..128
                # slot1: den(2pr+1) @p0..64, o(2pr+1) @p64..128
                oTn = work.tile([128, 4, LRP], F16, tag="oTn", name="oTn",
                                bufs=1)
                for pr in range(4):
                    dn = work.tile([128, 2, PC], F32, tag="dn", bufs=1)
                    nc.sync.dma_start(out=dn[0:64, 0, :],
                                      in_=oT[pr][64:128, 0, :])
                    nc.sync.dma_start(out=dn[64:128, 1, :],
                                      in_=oT[pr][0:64, 1, :])
                    nc.vector.reciprocal(out=dn[0:64, 0, :],
                                         in_=dn[0:64, 0, :])
                    nc.vector.reciprocal(out=dn[64:128, 1, :],
                                         in_=dn[64:128, 1, :])
                    nc.vector.tensor_mul(out=oTn[0:64, pr, 0:PC],
                                         in0=oT[pr][0:64, 0, :],
                                         in1=dn[0:64, 0, :])
                    nc.vector.tensor_mul(out=oTn[64:128, pr, 0:PC],
                                         in0=oT[pr][64:128, 1, :],
                                         in1=dn[64:128, 1, :])

                # ---- out projection + residual ----
                with tc.tile_pool(name="pF", bufs=2, space="PSUM") as pF:
                    for g in range(5):
                        real = RT_REAL[g]
                        lo = 128 * g
                        ps = pF.tile([128, 512], F32, tag="psF")
                        for q in range(4):
                            nc.tensor.matmul(
                                ps[:real],
                                oTn[:, q, lo:lo + real],
                                ow_sb[:, q, :],
                                start=(q == 0), stop=(q == 3))
                        nc.vector.tensor_add(out=ps[:real], in0=ps[:real],
                                             in1=lnv_sb[:real, 5, :])
                        nc.vector.tensor_add(out=seq[:real, g, :],
                                             in0=seq[:real, g, :],
                                             in1=ps[:real])

                # ---- LN2 + MLP ----
                h2T = _transpose(_ln(2, 3, "h1"), "hT")
                with tc.tile_pool(name="pG", bufs=3, space="PSUM") as pG:
                    for (rlo, rcnt) in ((0, 256), (256, 264)):
                        gT = work.tile([128, 16, 264], F16, tag="gT",
                                       name="gT", bufs=1)
                        for m in range(16):
                            ps = pG.tile([128, 512], F32, tag="psG")
                            for e in range(4):
                                nc.tensor.matmul(
                                    ps[:, :rcnt],
                                    w1_sb[:, e, 128 * m:128 * (m + 1)],
                                    h2T[:, e, rlo:rlo + rcnt],
                                    start=(e == 0), stop=(e == 3))
                            nc.scalar.activation(out=gT[:, m, :rcnt],
                                                 in_=ps[:, :rcnt],
                                                 func=AF.Gelu_apprx_tanh,
                                                 bias=pb24_sb[:, 8 + m:9 + m],
                                                 scale=1.0)
                        for (g, po, cnt, off) in _row_spans(rlo, rcnt):
                            real = min(cnt, max(0, RT_REAL[g] - po))
                            if real <= 0:
                                continue
                            ps = pG.tile([128, 512], F32, tag="psM")
                            for cd in range(16):
                                nc.tensor.matmul(
                                    ps[:real],
                                    gT[:, cd, off:off + real],
                                    w2_sb[:, cd, :],
                                    start=(cd == 0), stop=(cd == 15))
                            nc.vector.tensor_add(out=ps[:real], in0=ps[:real],
                                                 in1=lnv_sb[:real, 6, :])
                            nc.vector.tensor_add(out=seq[po:po + real, g, :],
                                                 in0=seq[po:po + real, g, :],
                                                 in1=ps[:real])

            # ================= output =================
            with tc.tile_pool(name="tail", bufs=1) as tp, \
                 tc.tile_pool(name="tailp", bufs=2, space="PSUM") as tpp:
                ngb = tp.tile([128, 2, E], F32, tag="ngb")
                nc.sync.dma_start(out=ngb[:, 0, :],
                                  in_=ng_d.partition_broadcast(128))
                nc.sync.dma_start(out=ngb[:, 1, :],
                                  in_=nb_d.partition_broadcast(128))
                hf = work.tile([128, 5, E], F16, tag="h1", name="hf2", bufs=1)
                for g in range(5):
                    cnt = 16 if g == 4 else 128
                    stats = work.tile([128, 6], F32, tag="f_st")
                    nc.vector.bn_stats(out=stats[:cnt], in_=seq[:cnt, g, :])
                    mv = work.tile([128, 2], F32, tag="f_mv")
                    nc.vector.bn_aggr(out=mv[:cnt], in_=stats[:cnt])
                    rs = work.tile([128, 1], F32, tag="f_rs")
                    nc.scalar.activation(out=rs[:cnt], in_=mv[:cnt, 1:2],
                                         func=AF.Sqrt, bias=eps_t[:cnt],
                                         scale=1.0)
                    nc.vector.reciprocal(out=rs[:cnt], in_=rs[:cnt])
                    y = work.tile([128, E], F32, tag="f_y")
                    nc.vector.tensor_scalar(out=y[:cnt], in0=seq[:cnt, g, :],
                                            scalar1=mv[:cnt, 0:1],
                                            scalar2=rs[:cnt],
                                            op0=mybir.AluOpType.subtract,
                                            op1=mybir.AluOpType.mult)
                    nc.vector.tensor_mul(out=y[:cnt], in0=y[:cnt],
                                         in1=ngb[:cnt, 0, :])
                    nc.vector.tensor_add(out=hf[:cnt, g, :], in0=y[:cnt],
                                         in1=ngb[:cnt, 1, :])
                hfT = tp.tile([128, 4, LRP], F16, tag="hfT", name="hfT")
                for g in range(5):
                    pcnt = 16 if g == 4 else 128
                    for e in range(4):
                        nc.scalar.dma_start_transpose(
                            out=hfT[:, e, 128 * g:128 * g + pcnt],
                            in_=hf[0:pcnt, g, 128 * e:128 * (e + 1)])
                hq = tp.tile([128, 4, 264], F16, tag="hq", name="hq")
                for e in range(4):
                    x = hfT[0:128, e, 32:65]
                    src = _mk_ap(x, [[_pstride(x), 128], [JJ, 8], [1, 33]])
                    nc.sync.dma_start(out=hq[:, e, :], in_=src)

                opw_sb = tp.tile([128, 4, D], F16, tag="opw")
                nc.sync.dma_start(
                    out=opw_sb, in_=opw_d.rearrange("(e p) d -> p e d", p=128))
                opb_bc = tp.tile([128, D], F32, tag="opb")
                nc.sync.dma_start(out=opb_bc, in_=opb_d.partition_broadcast(128))

                for (mlo, mcnt) in ((0, 128), (128, 128), (256, 8)):
                    ot = tp.tile([128, D], F32, tag="otile")
                    for nn in range(2):
                        ps = tpp.tile([128, 512], F32, tag="psO")
                        for e in range(4):
                            nc.tensor.matmul(
                                ps[:mcnt],
                                hq[:, e, mlo:mlo + mcnt],
                                opw_sb[:, e, 512 * nn:512 * (nn + 1)],
                                start=(e == 0), stop=(e == 3))
                        nc.vector.tensor_add(
                            out=ot[:mcnt, 512 * nn:512 * (nn + 1)],
                            in0=ps[:mcnt],
                            in1=opb_bc[:mcnt, 512 * nn:512 * (nn + 1)])
                    nc.sync.dma_start(out=out_d[mlo:mlo + mcnt, :],
                                      in_=ot[:mcnt])

    nc.compile()
    nc.m = get_hw_module(nc.m)
    return nc


# ---------------- host side ----------------

def _ln_np(x, eps=1e-5):
    m = x.mean(-1, keepdims=True)
    v = ((x - m) ** 2).mean(-1, keepdims=True)
    return (x - m) / np.sqrt(v + eps)


def _pack16(w, blocks=4):
    """[K, M] -> [128, K//128, M] with rows d = 128e + p."""
    return np.ascontiguousarray(
        w.reshape(blocks, 128, -1).transpose(1, 0, 2)).astype(np.float16)


def make_in_maps(inputs):
    f = {n: np.asarray(v) for n, v in inputs.items()}
    z = f["z_past"][0]
    code = f["code_embeddings"][0]
    q = f["query_embed"][0]
    pos = f["pos_embed"][0]

    qkw16 = np.zeros((DEPTH, 128, 4, 1024), np.float16)
    vw16 = np.zeros((DEPTH, 128, 4, 512), np.float16)
    ow16 = np.zeros((DEPTH, 128, 4, 512), np.float16)
    w1_16 = np.zeros((DEPTH, 128, 4, 2048), np.float16)
    w2_16 = np.zeros((DEPTH, 128, 16, 512), np.float16)
    pb24 = np.zeros((DEPTH, 128, 24), np.float32)
    lnvec = np.zeros((DEPTH, 7, 512), np.float32)
    for li in range(DEPTH):
        qkw16[li] = _pack16(f["qkv_w"][li][:, :1024])
        vw16[li] = _pack16(f["qkv_w"][li][:, 1024:])
        ow16[li] = _pack16(f["out_w"][li])
        w1_16[li] = _pack16(f["mlp_w1"][li])
        w2_16[li] = _pack16(f["mlp_w2"][li], blocks=16)
        pb24[li, :, :8] = f["qkv_b"][li][:1024].reshape(8, 128).T
        pb24[li, :, 8:] = f["mlp_b1"][li].reshape(16, 128).T
        lnvec[li] = np.stack([
            f["ln1_g"][li], f["ln1_b"][li], f["ln2_g"][li], f["ln2_b"][li],
            f["qkv_b"][li][1024:], f["out_b"][li], f["mlp_b2"][li]])

    shared = {
        "patch_w": f["patch_w"].astype(np.float16),
        "patch_b": f["patch_b"].astype(np.float32),
        "qkw16": qkw16, "vw16": vw16, "ow16": ow16,
        "w1_16": w1_16, "w2_16": w2_16,
        "pb24": pb24, "lnvec": lnvec,
        "norm_g": f["norm_g"].astype(np.float32),
        "norm_b": f["norm_b"].astype(np.float32),
        "oproj_w": f["oproj_w"].astype(np.float16),
        "oproj_b": f["oproj_b"].astype(np.float32),
    }

    c_proj = _ln_np(code.astype(np.float32) @ f["code_w"] + f["code_b"])

    in_maps = []
    for k in range(NC_):
        zk = z[:, k::8, :].reshape(256, D)
        zT = np.ascontiguousarray(zk.T).astype(np.float16)
        posz = np.zeros((256, E), np.float32)
        rest = np.zeros((T * 33, E), np.float32)
        for c in range(T):
            for jj in range(32):
                posz[32 * c + jj] = pos[515 * c + 8 * jj + k]
            for jj in range(32, JJ):
                p = 8 * jj + k
                ri = 33 * c + (jj - 32)
                if p < 259:
                    rest[ri] = c_proj[c, p - 256] + pos[515 * c + p]
                elif p < CHUNK:
                    rest[ri] = q[p - 259] + pos[515 * c + p]
        rowmask = np.zeros(640, np.float32)
        for l in range(LR):
            rowmask[l] = 1.0 if (8 * (l % JJ) + k) < CHUNK else 0.0
        m = dict(shared)
        m["zT"] = zT
        m["posz"] = posz
        m["rest"] = rest
        m["rowmask"] = rowmask
        in_maps.append(m)
    return in_maps


def unshard_output(results, dtype):
    out = np.zeros((1, T, N, D), np.float32)
    for k in range(NC_):
        pred = results[k]["out"]
        for c in range(T):
            for i2 in range(33):
                p = 256 + 8 * i2 + k
                if 259 <= p < CHUNK:
                    out[0, c, p - 259] = pred[33 * c + i2]
    return out.astype(dtype)


_PROG_LOCK = threading.Lock()
_PROG = None


def _get_prog():
    global _PROG
    with _PROG_LOCK:
        if _PROG is None:
            _PROG = build_program()
    return _PROG


def kernel(**inputs):
    nc = _get_prog()
    in_maps = make_in_maps(inputs)
    res = run_bass_kernel_spmd(nc, in_maps, list(range(NC_)))
    return unshard_output(res.results, np.asarray(inputs["z_past"]).dtype)


if __name__ == "__main__":
    nc = build_program()
    print("program built ok")


# revision 37
# speedup vs baseline: 1.1700x; 1.0559x over previous
"""Trainium2 Bass kernel for nn_BatchedVQLAMDecoder (8-core SPMD).

Sharding: mod-8 interleave of the 4120-token sequence (8 chunks of 515,
padded to 520). Core k owns padded rows p == k (mod 8) of every chunk.
Block-causal mask at chunk granularity => uniform SPMD.

v3 (all fp16 matmuls):
- 4 AllGathers per layer (2 chunks each) instead of 8 small ones.
- V shipped in head-pair layout [V_even|ones|V_odd]x4 (768 wide) so the
  gather is 3 contiguous DMAs and the softmax denominators come out of
  the same AV matmul (ones block), aligned for the normalize multiply.
- col kappa = 64r + l (l<64), extras l=64 at 512+r: contiguous gathers.
- merged weight/bias DMAs; oT accumulation split across DVE and Pool.
"""

import threading

import numpy as np

import bass_rust
import concourse.bass as bass
import concourse.tile as tile
from concourse import bacc, mybir
from concourse.bass_utils import run_bass_kernel_spmd
from concourse.bass_interp import get_hw_module

F32 = mybir.dt.float32
F16 = mybir.dt.float16
U8 = mybir.dt.uint8
AF = mybir.ActivationFunctionType

T, N, D, CDim, E, H, DEPTH = 8, 256, 1024, 128, 512, 8, 3
Dh = E // H                   # 64
CHUNK = 2 * N + 3             # 515
JJ = 65                       # local rows per (core, chunk)
PC = 8 * JJ                   # padded chunk = 520
LR = T * JJ                   # local rows per core = 520
LRP = 528                     # padded to mult-16 for dma transpose
NC_ = 8
SCALE = 1.0 / np.sqrt(Dh)

KBYTES = E * JJ * 2           # fp16 K^T slab bytes per chunk = 66560
VW = 768                      # V slab width: 4x [V_even(64)|ones(64)|V_odd(64)]
VBYTES = JJ * VW * 2          # fp16 V slab bytes per chunk = 99840
CKVB = KBYTES + VBYTES        # 166400
PAIRB = 2 * CKVB              # AG payload: 2 chunks

RT_REAL = [128, 128, 128, 128, 8]
CS = [(0, 128), (128, 128), (256, 128), (384, 128), (512, 8)]


def _row_spans(lo, cnt):
    out = []
    done = 0
    while done < cnt:
        g = lo + done
        t = g // 128
        po = g % 128
        c = min(128 - po, cnt - done)
        out.append((t, po, c, done))
        done += c
    return out


def _mk_ap(x, dims):
    c = x.copy()
    c.ap = bass_rust.VecI64Pair([[int(a), int(b)] for a, b in dims])
    return c


def _pstride(x):
    return int([list(p) for p in x.ap][0][0])


def build_program():
    nc = bacc.Bacc("TRN2", target_bir_lowering=False, debug=False,
                   num_devices=NC_)

    def inp(name, shape, dt):
        return nc.dram_tensor(name, list(shape), dt, kind="ExternalInput").ap()

    zT_d = inp("zT", [D, 2 * CDim], F16)
    posz_d = inp("posz", [2 * CDim, E], F32)
    rest_d = inp("rest", [T * 33, E], F32)
    rmask_d = inp("rowmask", [640], F32)
    pw_d = inp("patch_w", [D, E], F16)
    pb_d = inp("patch_b", [E], F32)
    qkw_d = inp("qkw16", [DEPTH, 128, 4, 2 * E], F16)
    vw_d = inp("vw16", [DEPTH, 128, 4, E], F16)
    ow_d = inp("ow16", [DEPTH, 128, 4, E], F16)
    w1_d = inp("w1_16", [DEPTH, 128, 4, 4 * E], F16)
    w2_d = inp("w2_16", [DEPTH, 128, 16, E], F16)
    pb24_d = inp("pb24", [DEPTH, 128, 24], F32)
    lnv_d = inp("lnvec", [DEPTH, 7, E], F32)
    ng_d = inp("norm_g", [E], F32)
    nb_d = inp("norm_b", [E], F32)
    opw_d = inp("oproj_w", [E, D], F16)
    opb_d = inp("oproj_b", [D], F32)

    out_d = nc.dram_tensor("out", [T * 33, D], F32, kind="ExternalOutput").ap()

    kv_loc = [nc.dram_tensor(f"kv_loc{h}", [PAIRB], U8).ap() for h in range(4)]
    kv_g = [nc.dram_tensor(f"kv_g{h}", [NC_, PAIRB], U8,
                           addr_space="Shared").ap() for h in range(4)]

    with tile.TileContext(nc) as tc:
        import contextlib
        ctx = contextlib.ExitStack()
        with ctx:
            persist = ctx.enter_context(tc.tile_pool(name="persist", bufs=1))
            work = ctx.enter_context(tc.tile_pool(name="work", bufs=2))
            wts = ctx.enter_context(tc.tile_pool(name="wts", bufs=1))

            eps_t = persist.tile([128, 1], F32, tag="eps")
            nc.vector.memset(eps_t, 1e-5)
            rmask_sb = persist.tile([128, 5], F32, tag="rmask")
            nc.sync.dma_start(
                out=rmask_sb, in_=rmask_d.rearrange("(g p) -> p g", p=128))
            # V chunk tiles (2 parity): per head pair pr:
            # [V(2pr) 64 | ones 64 | V(2pr+1) 64]
            vc = [persist.tile([128, 5, VW], F16, tag=f"vc{par}",
                               name=f"vc{par}") for par in range(2)]
            seq = persist.tile([128, 5, E], F32, tag="seq", name="seq")
            nc.vector.memset(seq, 0.0)

            # ------------- front end -------------
            with tc.tile_pool(name="front", bufs=1) as fp, \
                 tc.tile_pool(name="frontp", bufs=2, space="PSUM") as fpp:
                pb_bc = fp.tile([128, E], F32, tag="pb_bc")
                nc.sync.dma_start(out=pb_bc, in_=pb_d.partition_broadcast(128))
                zT_sb = fp.tile([128, 8, 256], F16, tag="zT")
                nc.sync.dma_start(
                    out=zT_sb, in_=zT_d.rearrange("(c p) n -> p c n", p=128))
                pw_sb = fp.tile([128, 8, E], F16, tag="pw")
                nc.sync.dma_start(
                    out=pw_sb, in_=pw_d.rearrange("(c p) n -> p c n", p=128))
                posz_sb = fp.tile([128, 2, E], F32, tag="posz")
                nc.sync.dma_start(
                    out=posz_sb, in_=posz_d.rearrange("(t p) e -> p t e", p=128))

                for zt in range(2):
                    ps = fpp.tile([128, E], F32, tag="psZ")
                    for cd in range(8):
                        nc.tensor.matmul(ps, zT_sb[:, cd, 128 * zt:128 * (zt + 1)],
                                         pw_sb[:, cd, :],
                                         start=(cd == 0), stop=(cd == 7))
                    zf = fp.tile([128, E], F32, tag="zf")
                    nc.vector.tensor_add(out=zf, in0=ps, in1=pb_bc)
                    stats = fp.tile([128, 6], F32, tag="zstats")
                    nc.vector.bn_stats(out=stats, in_=zf)
                    mv = fp.tile([128, 2], F32, tag="zmv")
                    nc.vector.bn_aggr(out=mv, in_=stats)
                    rs = fp.tile([128, 1], F32, tag="zrs")
                    nc.scalar.activation(out=rs, in_=mv[:, 1:2], func=AF.Sqrt,
                                         bias=eps_t, scale=1.0)
                    nc.vector.reciprocal(out=rs, in_=rs)
                    zn = fp.tile([128, E], F32, tag="zn")
                    nc.vector.tensor_scalar(out=zn, in0=zf,
                                            scalar1=mv[:, 0:1], scalar2=rs,
                                            op0=mybir.AluOpType.subtract,
                                            op1=mybir.AluOpType.mult)
                    nc.vector.tensor_add(out=zn, in0=zn, in1=posz_sb[:, zt, :])
                    for cb in range(4):
                        c = 4 * zt + cb
                        for (g, po, cnt, off) in _row_spans(JJ * c, 32):
                            nc.sync.dma_start(
                                out=seq[po:po + cnt, g, :],
                                in_=zn[32 * cb + off:32 * cb + off + cnt, :])
                for c in range(T):
                    for (g, po, cnt, off) in _row_spans(JJ * c + 32, 33):
                        nc.sync.dma_start(
                            out=seq[po:po + cnt, g, :],
                            in_=rest_d[33 * c + off:33 * c + off + cnt, :])

            # ================= layers =================
            for li in range(DEPTH):
                lnv_sb = wts.tile([128, 7, E], F32, tag="lnv")
                nc.sync.dma_start(
                    out=lnv_sb,
                    in_=lnv_d[li].rearrange("v e -> (v e)")
                    .partition_broadcast(128).rearrange("p (v e) -> p v e", v=7))
                pb24_sb = wts.tile([128, 24], F32, tag="pb24")
                nc.sync.dma_start(out=pb24_sb, in_=pb24_d[li])
                qkw_sb = wts.tile([128, 4, 2 * E], F16, tag="qkw", bufs=2)
                nc.sync.dma_start(out=qkw_sb, in_=qkw_d[li])
                vw_sb = wts.tile([128, 4, E], F16, tag="vw")
                nc.sync.dma_start(out=vw_sb, in_=vw_d[li])
                ow_sb = wts.tile([128, 4, E], F16, tag="ow")
                nc.sync.dma_start(out=ow_sb, in_=ow_d[li])
                w1_sb = wts.tile([128, 4, 4 * E], F16, tag="w1")
                nc.sync.dma_start(out=w1_sb, in_=w1_d[li])
                w2_sb = wts.tile([128, 16, E], F16, tag="w2")
                nc.sync.dma_start(out=w2_sb, in_=w2_d[li])

                def _ln(gi, bi, tag):
                    h16 = work.tile([128, 5, E], F16, tag=tag, name=tag,
                                    bufs=1)
                    for g in range(5):
                        cnt = 16 if g == 4 else 128
                        stats = work.tile([128, 6], F32, tag="ln_st")
                        nc.vector.bn_stats(out=stats[:cnt], in_=seq[:cnt, g, :])
                        mv = work.tile([128, 2], F32, tag="ln_mv")
                        nc.vector.bn_aggr(out=mv[:cnt], in_=stats[:cnt])
                        rs = work.tile([128, 1], F32, tag="ln_rs")
                        nc.scalar.activation(out=rs[:cnt], in_=mv[:cnt, 1:2],
                                             func=AF.Sqrt, bias=eps_t[:cnt],
                                             scale=1.0)
                        nc.vector.reciprocal(out=rs[:cnt], in_=rs[:cnt])
                        y = work.tile([128, E], F32, tag="ln_y")
                        nc.vector.tensor_scalar(out=y[:cnt], in0=seq[:cnt, g, :],
                                                scalar1=mv[:cnt, 0:1],
                                                scalar2=rs[:cnt],
                                                op0=mybir.AluOpType.subtract,
                                                op1=mybir.AluOpType.mult)
                        veng = nc.vector if g % 2 == 0 else nc.gpsimd
                        veng.tensor_mul(out=y[:cnt], in0=y[:cnt],
                                        in1=lnv_sb[:cnt, gi, :])
                        veng.tensor_add(out=h16[:cnt, g, :], in0=y[:cnt],
                                        in1=lnv_sb[:cnt, bi, :])
                    return h16

                def _transpose(h16, tag):
                    hT = work.tile([128, 4, LRP], F16, tag=tag, name=tag, bufs=1)
                    for g in range(5):
                        pcnt = 16 if g == 4 else 128
                        for e in range(4):
                            nc.sync.dma_start_transpose(
                                out=hT[:, e, 128 * g:128 * g + pcnt],
                                in_=h16[0:pcnt, g, 128 * e:128 * (e + 1)])
                    return hT

                h1T = _transpose(_ln(0, 1, "h1"), "hT")

                qkQ = work.tile([128, 4, LRP], F16, tag="qkQ", name="qkQ",
                                bufs=1)
                qkK = work.tile([128, 4, LRP], F16, tag="qkK", name="qkK",
                                bufs=1)

                def _qk_tiles(pB, ms):
                    for m in ms:
                        for (no, nw) in ((0, 512), (512, 8)):
                            ps = pB.tile([128, 512], F32, tag="psqk")
                            for e in range(4):
                                nc.tensor.matmul(
                                    ps[:, :nw],
                                    qkw_sb[:, e, 128 * m:128 * (m + 1)],
                                    h1T[:, e, no:no + nw],
                                    start=(e == 0), stop=(e == 3))
                            dst = (qkQ[:, m, no:no + nw] if m < 4 else
                                   qkK[:, m - 4, no:no + nw])
                            nc.vector.tensor_scalar_add(
                                out=dst, in0=ps[:, :nw],
                                scalar1=pb24_sb[:, m:m + 1])

                with tc.tile_pool(name="pB", bufs=3, space="PSUM") as pB:
                    _qk_tiles(pB, [4, 5, 6, 7])      # K^T first
                    # V natural [row%128, g, VW] f16, head-pair layout
                    v16 = work.tile([128, 5, VW], F16, tag="v16", name="v16",
                                    bufs=1)
                    for g in range(5):
                        real = RT_REAL[g]
                        ps = pB.tile([128, 512], F32, tag="psv")
                        for e in range(4):
                            nc.tensor.matmul(
                                ps[:real],
                                h1T[:, e, 128 * g:128 * g + real],
                                vw_sb[:, e, :],
                                start=(e == 0), stop=(e == 3))

                        def _s4(x, st, real=real):
                            return _mk_ap(x, [[_pstride(x), real],
                                              [st, 4], [1, 64]])
                        nc.vector.memset(_s4(v16[:real, g, 64:128], 192), 1.0)
                        for par in range(2):
                            nc.vector.tensor_add(
                                out=_s4(v16[:real, g, 128 * par:
                                            128 * par + 64], 192),
                                in0=_s4(ps[:real, 64 * par:64 * par + 64], 128),
                                in1=_s4(lnv_sb[:real, 4, 64 * par:
                                               64 * par + 64], 128))
                        nc.vector.tensor_scalar_mul(
                            out=v16[:real, g, :], in0=v16[:real, g, :],
                            scalar1=rmask_sb[:real, g:g + 1])

                    # ship K^T/V per chunk; 4 AGs per layer (2 chunks each)
                    for c in range(T):
                        grp, rel = c // 2, c % 2
                        base = rel * CKVB
                        kdst = (kv_loc[grp][base:base + KBYTES]
                                .bitcast(F16)
                                .rearrange("(g p j) -> p g j", g=4, p=128))
                        nc.sync.dma_start(out=kdst,
                                          in_=qkK[:, :, JJ * c:JJ * c + JJ])
                        vdst = (kv_loc[grp][base + KBYTES:base + CKVB]
                                .bitcast(F16)
                                .rearrange("(l e) -> l e", e=VW))
                        for (g, po, cnt, off) in _row_spans(JJ * c, JJ):
                            nc.sync.dma_start(
                                out=vdst[off:off + cnt, :],
                                in_=v16[po:po + cnt, g, :])
                        if rel == 1:
                            nc.gpsimd.collective_compute(
                                "AllGather", mybir.AluOpType.bypass,
                                replica_groups=[list(range(NC_))],
                                ins=[kv_loc[grp][:]], outs=[kv_g[grp][:, :]])
                    _qk_tiles(pB, [0, 1, 2, 3])      # Q^T overlaps AGs

                # ---- attention ----
                oT = [work.tile([128, 2, PC], F32, tag=f"oT{pr}",
                                name=f"oT{pr}", bufs=1) for pr in range(4)]
                with tc.tile_pool(name="pS", bufs=2, space="PSUM") as pS, \
                     tc.tile_pool(name="pO", bufs=2, space="PSUM") as pO:
                    for cpr in range(T):
                        grp, rel = cpr // 2, cpr % 2
                        base = rel * CKVB
                        ktc = work.tile([128, 4, PC], F16, tag="ktc", bufs=2)
                        ksrc = (kv_g[grp][:, base:base + KBYTES]
                                .bitcast(F16)
                                .rearrange("r (g p j) -> p g r j", g=4, p=128))
                        for qb in range(4):
                            nc.sync.dma_start(
                                out=ktc[:, qb, 0:512],
                                in_=ksrc[:, qb:qb + 1, :, 0:64].squeeze(1))
                            nc.sync.dma_start(
                                out=ktc[:, qb, 512:520],
                                in_=ksrc[:, qb:qb + 1, :, 64:65]
                                .squeeze(3).squeeze(1))
                        vcp = vc[cpr % 2]
                        vsrc = (kv_g[grp][:, base + KBYTES:base + CKVB]
                                .bitcast(F16)
                                .rearrange("r (l e) -> r l e", e=VW))
                        # col kappa = 64r + l (l<64); extras l=64 at 512+r
                        for par0 in range(2):
                            dst0 = vcp[64 * par0:64 * par0 + 64, 0, :]
                            d3 = _mk_ap(dst0, [[_pstride(dst0), 64],
                                               [VW, 4], [1, VW]])
                            nc.sync.dma_start(
                                out=d3,
                                in_=vsrc[par0:8:2, 0:64, :]
                                .rearrange("r l e -> l r e"))
                        nc.sync.dma_start(out=vcp[0:8, 4, :],
                                          in_=vsrc[:, 64:65, :].squeeze(1))

                        passes = ([(0, 512), (512, 8)] if cpr == 0
                                  else [(JJ * cpr, LR - JJ * cpr)])
                        for (rlo, rcnt) in passes:
                            for pr in range(4):
                                poT = pO.tile([128, 2, 512], F32, tag="poT")
                                for s in range(5):
                                    lo, cw = CS[s]
                                    psq = pS.tile([128, 2, 512], F32, tag="psq")
                                    for hh in range(2):
                                        nc.tensor.matmul(
                                            psq[:cw, hh, :rcnt],
                                            ktc[64 * hh:64 * hh + 64,
                                                pr, lo:lo + cw],
                                            qkQ[64 * hh:64 * hh + 64,
                                                pr, rlo:rlo + rcnt],
                                            start=True, stop=True)
                                    eq = work.tile([128, 2, PC], F16,
                                                   tag="eq", bufs=2)
                                    nc.scalar.activation(
                                        out=eq[:cw, :, :rcnt],
                                        in_=psq[:cw, :, :rcnt],
                                        func=AF.Exp, scale=float(SCALE))
                                    for hh in range(2):
                                        off = 192 * pr + 64 * hh
                                        nc.tensor.matmul(
                                            poT[:, hh, :rcnt],
                                            vcp[0:cw, s, off:off + 128],
                                            eq[:cw, hh, :rcnt],
                                            start=(s == 0), stop=(s == 4))
                                eng = nc.vector
                                if cpr == 0:
                                    eng.tensor_copy(
                                        out=oT[pr][:, :, rlo:rlo + rcnt],
                                        in_=poT[:, :, :rcnt])
                                else:
                                    eng.tensor_add(
                                        out=oT[pr][:, :, rlo:rlo + rcnt],
                                        in0=oT[pr][:, :, rlo:rlo + rcnt],
                                        in1=poT[:, :, :rcnt])

                # ---- normalize (denominators already in oT) ----
                # slot0: o(2pr) @p0..64, den(2pr) @p64..128
                # slot1: den(2pr+1) @p0..64, o(2pr+1) @p64..128
                oTn = work.tile([128, 4, LRP], F16, tag="oTn", name="oTn",
                                bufs=1)
                for pr in range(4):
                    eng = nc.vector if pr % 2 == 0 else nc.gpsimd
                    dn = work.tile([128, 2, PC], F32, tag="dn", bufs=2)
                    nc.sync.dma_start(out=dn[0:64, 0, :],
                                      in_=oT[pr][64:128, 0, :])
                    nc.sync.dma_start(out=dn[64:128, 1, :],
                                      in_=oT[pr][0:64, 1, :])
                    nc.vector.reciprocal(out=dn[0:64, 0, :],
                                         in_=dn[0:64, 0, :])
                    nc.vector.reciprocal(out=dn[64:128, 1, :],
                                         in_=dn[64:128, 1, :])
                    eng.tensor_mul(out=oTn[0:64, pr, 0:PC],
                                   in0=oT[pr][0:64, 0, :],
                                   in1=dn[0:64, 0, :])
                    eng.tensor_mul(out=oTn[64:128, pr, 0:PC],
                                   in0=oT[pr][64:128, 1, :],
                                   in1=dn[64:128, 1, :])

                # ---- out projection + residual ----
                with tc.tile_pool(name="pF", bufs=2, space="PSUM") as pF:
                    for g in range(5):
                        real = RT_REAL[g]
                        lo = 128 * g
                        ps = pF.tile([128, 512], F32, tag="psF")
                        for q in range(4):
                            nc.tensor.matmul(
                                ps[:real],
                                oTn[:, q, lo:lo + real],
                                ow_sb[:, q, :],
                                start=(q == 0), stop=(q == 3))
                        nc.vector.tensor_add(out=ps[:real], in0=ps[:real],
                                             in1=lnv_sb[:real, 5, :])
                        nc.vector.tensor_add(out=seq[:real, g, :],
                                             in0=seq[:real, g, :],
                                             in1=ps[:real])

                # ---- LN2 + MLP ----
                h2T = _transpose(_ln(2, 3, "h1"), "hT")
                with tc.tile_pool(name="pG", bufs=3, space="PSUM") as pG:
                    for (rlo, rcnt) in ((0, 256), (256, 264)):
                        gT = work.tile([128, 16, 264], F16, tag="gT",
                                       name="gT", bufs=1)
                        for m in range(16):
                            ps = pG.tile([128, 512], F32, tag="psG")
                            for e in range(4):
                                nc.tensor.matmul(
                                    ps[:, :rcnt],
                                    w1_sb[:, e, 128 * m:128 * (m + 1)],
                                    h2T[:, e, rlo:rlo + rcnt],
                                    start=(e == 0), stop=(e == 3))
                            nc.scalar.activation(out=gT[:, m, :rcnt],
                                                 in_=ps[:, :rcnt],
                                                 func=AF.Gelu_apprx_tanh,
                                                 bias=pb24_sb[:, 8 + m:9 + m],
                                                 scale=1.0)
                        for (g, po, cnt, off) in _row_spans(rlo, rcnt):
                            real = min(cnt, max(0, RT_REAL[g] - po))
                            if real <= 0:
                                continue
                            ps = pG.tile([128, 512], F32, tag="psM")
                            for cd in range(16):
                                nc.tensor.matmul(
                                    ps[:real],
                                    gT[:, cd, off:off + real],
                                    w2_sb[:, cd, :],
                                    start=(cd == 0), stop=(cd == 15))
                            nc.vector.tensor_add(out=ps[:real], in0=ps[:real],
                                                 in1=lnv_sb[:real, 6, :])
                            nc.vector.tensor_add(out=seq[po:po + real, g, :],
                                                 in0=seq[po:po + real, g, :],
                                                 in1=ps[:real])

            # ================= output =================
            with tc.tile_pool(name="tail", bufs=1) as tp, \
                 tc.tile_pool(name="tailp", bufs=2, space="PSUM") as tpp:
                ngb = tp.tile([128, 2, E], F32, tag="ngb")
                nc.sync.dma_start(out=ngb[:, 0, :],
                                  in_=ng_d.partition_broadcast(128))
                nc.sync.dma_start(out=ngb[:, 1, :],
                                  in_=nb_d.partition_broadcast(128))
                hf = work.tile([128, 5, E], F16, tag="h1", name="hf2", bufs=1)
                for g in range(5):
                    cnt = 16 if g == 4 else 128
                    stats = work.tile([128, 6], F32, tag="f_st")
                    nc.vector.bn_stats(out=stats[:cnt], in_=seq[:cnt, g, :])
                    mv = work.tile([128, 2], F32, tag="f_mv")
                    nc.vector.bn_aggr(out=mv[:cnt], in_=stats[:cnt])
                    rs = work.tile([128, 1], F32, tag="f_rs")
                    nc.scalar.activation(out=rs[:cnt], in_=mv[:cnt, 1:2],
                                         func=AF.Sqrt, bias=eps_t[:cnt],
                                         scale=1.0)
                    nc.vector.reciprocal(out=rs[:cnt], in_=rs[:cnt])
                    y = work.tile([128, E], F32, tag="f_y")
                    nc.vector.tensor_scalar(out=y[:cnt], in0=seq[:cnt, g, :],
                                            scalar1=mv[:cnt, 0:1],
                                            scalar2=rs[:cnt],
                                            op0=mybir.AluOpType.subtract,
                                            op1=mybir.AluOpType.mult)
                    nc.vector.tensor_mul(out=y[:cnt], in0=y[:cnt],
                                         in1=ngb[:cnt, 0, :])
                    nc.vector.tensor_add(out=hf[:cnt, g, :], in0=y[:cnt],
                                         in1=ngb[:cnt, 1, :])
                hfT = tp.tile([128, 4, LRP], F16, tag="hfT", name="hfT")
                for g in range(5):
                    pcnt = 16 if g == 4 else 128
                    for e in range(4):
                        nc.sync.dma_start_transpose(
                            out=hfT[:, e, 128 * g:128 * g + pcnt],
                            in_=hf[0:pcnt, g, 128 * e:128 * (e + 1)])
                hq = tp.tile([128, 4, 264], F16, tag="hq", name="hq")
                for e in range(4):
                    x = hfT[0:128, e, 32:65]
                    src = _mk_ap(x, [[_pstride(x), 128], [JJ, 8], [1, 33]])
                    nc.sync.dma_start(out=hq[:, e, :], in_=src)

                opw_sb = tp.tile([128, 4, D], F16, tag="opw")
                nc.sync.dma_start(
                    out=opw_sb, in_=opw_d.rearrange("(e p) d -> p e d", p=128))
                opb_bc = tp.tile([128, D], F32, tag="opb")
                nc.sync.dma_start(out=opb_bc, in_=opb_d.partition_broadcast(128))

                for (mlo, mcnt) in ((0, 128), (128, 128), (256, 8)):
                    ot = tp.tile([128, D], F32, tag="otile")
                    for nn in range(2):
                        ps = tpp.tile([128, 512], F32, tag="psO")
                        for e in range(4):
                            nc.tensor.matmul(
                                ps[:mcnt],
                                hq[:, e, mlo:mlo + mcnt],
                                opw_sb[:, e, 512 * nn:512 * (nn + 1)],
                                start=(e == 0), stop=(e == 3))
                        nc.vector.tensor_add(
                            out=ot[:mcnt, 512 * nn:512 * (nn + 1)],
                            in0=ps[:mcnt],
                            in1=opb_bc[:mcnt, 512 * nn:512 * (nn + 1)])
                    nc.sync.dma_start(out=out_d[mlo:mlo + mcnt, :],
                                      in_=ot[:mcnt])

    nc.compile()
    nc.m = get_hw_module(nc.m)
    return nc


# ---------------- host side ----------------

def _ln_np(x, eps=1e-5):
    m = x.mean(-1, keepdims=True)
    v = ((x - m) ** 2).mean(-1, keepdims=True)
    return (x - m) / np.sqrt(v + eps)


def _pack16(w, blocks=4):
    """[K, M] -> [128, K//128, M] with rows d = 128e + p."""
    return np.ascontiguousarray(
        w.reshape(blocks, 128, -1).transpose(1, 0, 2)).astype(np.float16)


def make_in_maps(inputs):
    f = {n: np.asarray(v) for n, v in inputs.items()}
    z = f["z_past"][0]
    code = f["code_embeddings"][0]
    q = f["query_embed"][0]
    pos = f["pos_embed"][0]

    qkw16 = np.zeros((DEPTH, 128, 4, 1024), np.float16)
    vw16 = np.zeros((DEPTH, 128, 4, 512), np.float16)
    ow16 = np.zeros((DEPTH, 128, 4, 512), np.float16)
    w1_16 = np.zeros((DEPTH, 128, 4, 2048), np.float16)
    w2_16 = np.zeros((DEPTH, 128, 16, 512), np.float16)
    pb24 = np.zeros((DEPTH, 128, 24), np.float32)
    lnvec = np.zeros((DEPTH, 7, 512), np.float32)
    for li in range(DEPTH):
        qkw16[li] = _pack16(f["qkv_w"][li][:, :1024])
        vw16[li] = _pack16(f["qkv_w"][li][:, 1024:])
        ow16[li] = _pack16(f["out_w"][li])
        w1_16[li] = _pack16(f["mlp_w1"][li])
        w2_16[li] = _pack16(f["mlp_w2"][li], blocks=16)
        pb24[li, :, :8] = f["qkv_b"][li][:1024].reshape(8, 128).T
        pb24[li, :, 8:] = f["mlp_b1"][li].reshape(16, 128).T
        lnvec[li] = np.stack([
            f["ln1_g"][li], f["ln1_b"][li], f["ln2_g"][li], f["ln2_b"][li],
            f["qkv_b"][li][1024:], f["out_b"][li], f["mlp_b2"][li]])

    shared = {
        "patch_w": f["patch_w"].astype(np.float16),
        "patch_b": f["patch_b"].astype(np.float32),
        "qkw16": qkw16, "vw16": vw16, "ow16": ow16,
        "w1_16": w1_16, "w2_16": w2_16,
        "pb24": pb24, "lnvec": lnvec,
        "norm_g": f["norm_g"].astype(np.float32),
        "norm_b": f["norm_b"].astype(np.float32),
        "oproj_w": f["oproj_w"].astype(np.float16),
        "oproj_b": f["oproj_b"].astype(np.float32),
    }

    c_proj = _ln_np(code.astype(np.float32) @ f["code_w"] + f["code_b"])

    in_maps = []
    for k in range(NC_):
        zk = z[:, k::8, :].reshape(256, D)
        zT = np.ascontiguousarray(zk.T).astype(np.float16)
        posz = np.zeros((256, E), np.float32)
        rest = np.zeros((T * 33, E), np.float32)
        for c in range(T):
            for jj in range(32):
                posz[32 * c + jj] = pos[515 * c + 8 * jj + k]
            for jj in range(32, JJ):
                p = 8 * jj + k
                ri = 33 * c + (jj - 32)
                if p < 259:
                    rest[ri] = c_proj[c, p - 256] + pos[515 * c + p]
                elif p < CHUNK:
                    rest[ri] = q[p - 259] + pos[515 * c + p]
        rowmask = np.zeros(640, np.float32)
        for l in range(LR):
            rowmask[l] = 1.0 if (8 * (l % JJ) + k) < CHUNK else 0.0
        m = dict(shared)
        m["zT"] = zT
        m["posz"] = posz
        m["rest"] = rest
        m["rowmask"] = rowmask
        in_maps.append(m)
    return in_maps


def unshard_output(results, dtype):
    out = np.zeros((1, T, N, D), np.float32)
    for k in range(NC_):
        pred = results[k]["out"]
        for c in range(T):
            for i2 in range(33):
                p = 256 + 8 * i2 + k
                if 259 <= p < CHUNK:
                    out[0, c, p - 259] = pred[33 * c + i2]
    return out.astype(dtype)


_PROG_LOCK = threading.Lock()
_PROG = None


def _get_prog():
    global _PROG
    with _PROG_LOCK:
        if _PROG is None:
            _PROG = build_program()
    return _PROG


def kernel(**inputs):
    nc = _get_prog()
    in_maps = make_in_maps(inputs)
    res = run_bass_kernel_spmd(nc, in_maps, list(range(NC_)))
    return unshard_output(res.results, np.asarray(inputs["z_past"]).dtype)


if __name__ == "__main__":
    nc = build_program()
    print("program built ok")
